# revision 5
# baseline (speedup 1.0000x reference)
import sys

if "/opt/trn_rl_repo" not in sys.path:
    sys.path.insert(0, "/opt/trn_rl_repo")

import numpy as np
from contextlib import ExitStack

import concourse.tile as tile
from concourse import bacc, mybir
from concourse import bass_utils

F32 = mybir.dt.float32
F32R = mybir.dt.float32r
AF = mybir.ActivationFunctionType
ALU = mybir.AluOpType
AX = mybir.AxisListType

B, C, L = 32, 128, 8192
N_CORES = 8
NB = B // N_CORES          # batches per core
CQ = C // 4
EPS = 1e-5
CH = 2048                  # P1 chunk
PCH = 1024                 # P2 chunk (2 PSUM banks)
OT = 512                   # P3 out tile (1 PSUM bank)

_BUILD_CACHE = {}


def _build(reps=1):
    if reps in _BUILD_CACHE:
        return _BUILD_CACHE[reps]

    nc = bacc.Bacc("TRN2", target_bir_lowering=False, debug=False)

    x_ap = nc.dram_tensor("x_dram", [NB, C, L], F32, kind="ExternalInput").ap()
    w_u_ap = nc.dram_tensor("w_u", [C, C], F32, kind="ExternalInput").ap()
    wsc_aps = [nc.dram_tensor(f"wsc{k}", [C, C], F32, kind="ExternalInput").ap() for k in range(3)]
    w2t_ap = nc.dram_tensor("w2t", [C, C], F32, kind="ExternalInput").ap()
    wfc1_ap = nc.dram_tensor("wfc1", [C, CQ], F32, kind="ExternalInput").ap()
    b1e_ap = nc.dram_tensor("b1e", [CQ, 1], F32, kind="ExternalInput").ap()
    wfc2_ap = nc.dram_tensor("wfc2", [CQ, C], F32, kind="ExternalInput").ap()
    b2_ap = nc.dram_tensor("b2", [C, 1], F32, kind="ExternalInput").ap()
    t2_ap = nc.dram_tensor("t2", [C, 1], F32, kind="ExternalInput").ap()
    ident_ap = nc.dram_tensor("ident", [C, C], F32, kind="ExternalInput").ap()
    taps_ap = nc.dram_tensor("taps", [1, 6], F32, kind="ExternalInput").ap()
    out_ap = nc.dram_tensor("out_dram", [NB, C, L], F32, kind="ExternalOutput").ap()

    with tile.TileContext(nc) as tc, ExitStack() as ctx:
        wpool = ctx.enter_context(tc.tile_pool(name="wpool", bufs=1))
        xr_pool = ctx.enter_context(tc.tile_pool(name="xr", bufs=2))
        x1_pool = ctx.enter_context(tc.tile_pool(name="x1", bufs=2))
        xin_pool = ctx.enter_context(tc.tile_pool(name="xin", bufs=2))
        scr_pool = ctx.enter_context(tc.tile_pool(name="scr", bufs=2))
        m_pool = ctx.enter_context(tc.tile_pool(name="mtile", bufs=2))
        out_pool = ctx.enter_context(tc.tile_pool(name="ot", bufs=4))
        st_pool = ctx.enter_context(tc.tile_pool(name="stats", bufs=2))
        row_pool = ctx.enter_context(tc.tile_pool(name="rows", bufs=2))
        w2a_pool = ctx.enter_context(tc.tile_pool(name="w2a", bufs=2))
        u_psp = ctx.enter_context(tc.tile_pool(name="u_ps", bufs=2, space="PSUM"))
        o_psp = ctx.enter_context(tc.tile_pool(name="o_ps", bufs=3, space="PSUM"))
        s_psp = ctx.enter_context(tc.tile_pool(name="s_ps", bufs=1, space="PSUM"))

        # ---- load + prep weights (once) ----
        def wload(nm, ap, shape):
            t = wpool.tile(shape, F32, tag=nm)
            nc.sync.dma_start(t[:], ap[:])
            return t

        w_u_f = wload("w_u_f", w_u_ap, [C, C])
        wsc_f = [wload(f"wsc{k}_f", wsc_aps[k], [C, C]) for k in range(3)]
        w2t_t = wload("w2t_t", w2t_ap, [C, C])
        wfc1_t = wload("wfc1_t", wfc1_ap, [C, CQ])
        b1e_t = wload("b1e_t", b1e_ap, [CQ, 1])
        wfc2_t = wload("wfc2_t", wfc2_ap, [CQ, C])
        b2_t = wload("b2_t", b2_ap, [C, 1])
        t2_t = wload("t2_t", t2_ap, [C, 1])
        ident_t = wload("ident_t", ident_ap, [C, C])
        taps_t = wload("taps_t", taps_ap, [1, 6])

        w_u_r = wpool.tile([C, C], F32R, tag="w_u_r")
        nc.vector.tensor_scalar(w_u_r[:], w_u_f[:], 0.0, None, ALU.add)
        wsc_r = []
        for k in range(3):
            t = wpool.tile([C, C], F32R, tag=f"wsc{k}_r")
            nc.vector.tensor_scalar(t[:], wsc_f[k][:], 0.0, None, ALU.add)
            wsc_r.append(t)
        ones_t = wpool.tile([1, C], F32, tag="ones_t")
        nc.vector.memset(ones_t[:], 1.0)

        # ---- per-batch pipeline ----
        for b in [b for _ in range(reps) for b in range(NB)]:
            xr = xr_pool.tile([C, L + 2], F32R, tag="xr")
            nc.vector.memset(xr[:, 0:1].bitcast(F32), 0.0)
            nc.vector.memset(xr[:, L + 1:L + 2].bitcast(F32), 0.0)

            sabs_p = st_pool.tile([C, L // CH], F32, tag="sabs_p")
            for q in range(L // CH):
                xin = xin_pool.tile([C, CH], F32, tag="xin")
                nc.sync.dma_start(xin[:], x_ap[b, :, q * CH:(q + 1) * CH])
                scr = scr_pool.tile([C, CH], F32, tag="scr")
                nc.scalar.activation(scr[:], xin[:], AF.Abs, accum_out=sabs_p[:, q:q + 1])
                nc.scalar.activation(xr[:, 1 + q * CH:1 + (q + 1) * CH], xin[:], AF.Copy)

            # channel-attention MLP -> threshold T
            sabs = st_pool.tile([C, 1], F32, tag="sabs")
            nc.vector.tensor_reduce(sabs[:], sabs_p[:], AX.X, ALU.add)
            h_ps = s_psp.tile([CQ, 1], F32, tag="s_ps")
            nc.tensor.matmul(h_ps[:], wfc1_t[:], sabs[:], start=True, stop=True)
            h_t = st_pool.tile([CQ, 1], F32, tag="h_t")
            nc.scalar.activation(h_t[:], h_ps[:], AF.Relu, bias=b1e_t[:], scale=1.0)
            y_ps = s_psp.tile([C, 1], F32, tag="s_ps")
            nc.tensor.matmul(y_ps[:], wfc2_t[:], h_t[:], start=True, stop=True)
            x12 = st_pool.tile([C, 1], F32, tag="x12")
            nc.scalar.activation(x12[:], y_ps[:], AF.Sigmoid, bias=b2_t[:], scale=1.0)
            tt = st_pool.tile([C, 1], F32, tag="tt")
            nc.vector.tensor_tensor(tt[:], sabs[:], x12[:], ALU.mult)
            tpos = st_pool.tile([C, 1], F32, tag="tpos")
            nc.vector.tensor_scalar(tpos[:], tt[:], 1.0 / L, None, ALU.mult)
            negt = st_pool.tile([C, 1], F32, tag="negt")
            nc.vector.tensor_scalar(negt[:], tt[:], -1.0 / L, None, ALU.mult)

            # x1 = max(min(xr, u+T), u-T) with u = (I+w1)@xr ; running sum via stt accum
            x1 = x1_pool.tile([C, L], F32R, tag="x1")
            ssum_p = st_pool.tile([C, L // PCH], F32, tag="ssum_p")
            for p in range(L // PCH):
                u_ps = u_psp.tile([C, PCH], F32, tag="u_ps")
                base = 1 + p * PCH
                nc.tensor.matmul(u_ps[:, 0:512], w_u_r[:], xr[:, base:base + 512],
                                 start=True, stop=True)
                nc.tensor.matmul(u_ps[:, 512:1024], w_u_r[:], xr[:, base + 512:base + 1024],
                                 start=True, stop=True)
                m_t = m_pool.tile([C, PCH], F32, tag="m_t")
                nc.vector.scalar_tensor_tensor(m_t[:], u_ps[:], tpos[:], xr[:, base:base + PCH],
                                               ALU.add, ALU.min)
                nc.vector.scalar_tensor_tensor(x1[:, p * PCH:(p + 1) * PCH], u_ps[:], negt[:],
                                               m_t[:], ALU.add, ALU.max,
                                               accum_out=ssum_p[:, p:p + 1])

            smax_p = st_pool.tile([C, L // CH], F32, tag="smax_p")
            for q in range(L // CH):
                scr2 = scr_pool.tile([C, CH], F32, tag="scr")
                nc.vector.tensor_scalar(scr2[:], x1[:, q * CH:(q + 1) * CH], 0.0, None,
                                        ALU.add, ALU.max, accum_out=smax_p[:, q:q + 1])

            # spatial attention a = sigmoid(conv3 over channel axis of [mean, max])
            s_x1 = st_pool.tile([C, 1], F32, tag="s_x1")
            nc.vector.tensor_reduce(s_x1[:], ssum_p[:], AX.X, ALU.add)
            mx = st_pool.tile([C, 1], F32, tag="mx")
            nc.vector.tensor_reduce(mx[:], smax_p[:], AX.X, ALU.max)

            mrow_ps = s_psp.tile([1, C], F32, tag="s_ps")
            nc.tensor.transpose(mrow_ps[:], s_x1[:], ident_t[:])
            meanrow = row_pool.tile([1, C + 2], F32, tag="meanrow")
            nc.vector.memset(meanrow[:], 0.0)
            nc.vector.tensor_copy(meanrow[:, 1:C + 1], mrow_ps[:])
            xrow_ps = s_psp.tile([1, C], F32, tag="s_ps")
            nc.tensor.transpose(xrow_ps[:], mx[:], ident_t[:])
            maxrow = row_pool.tile([1, C + 2], F32, tag="maxrow")
            nc.vector.memset(maxrow[:], 0.0)
            nc.vector.tensor_copy(maxrow[:, 1:C + 1], xrow_ps[:])

            lprev = row_pool.tile([1, C], F32, tag="l0")
            nc.vector.tensor_scalar(lprev[:], meanrow[:, 0:C], taps_t[0:1, 0:1], None, ALU.mult)
            chain = [
                (meanrow, 1, 1), (meanrow, 2, 2),
                (maxrow, 0, 3), (maxrow, 1, 4), (maxrow, 2, 5),
            ]
            for j, (row, off, k) in enumerate(chain):
                lnew = row_pool.tile([1, C], F32, tag=f"l{(j + 1) % 2}")
                nc.vector.scalar_tensor_tensor(lnew[:], row[:, off:off + C], taps_t[0:1, k:k + 1],
                                               lprev[:], ALU.mult, ALU.add)
                lprev = lnew
            arow = row_pool.tile([1, C], F32, tag="arow")
            nc.scalar.activation(arow[:], lprev[:], AF.Sigmoid)
            bc_ps = s_psp.tile([C, C], F32, tag="s_ps")
            nc.tensor.matmul(bc_ps[:], ones_t[:], arow[:], start=True, stop=True)
            w2a = w2a_pool.tile([C, C], F32R, tag="w2a")
            nc.vector.tensor_tensor(w2a[:], w2t_t[:], bc_ps[:], ALU.mult)

            # out = relu(a*(w2@x1) + bn2(conv3(x)) + t2)
            for i in range(L // OT):
                o_ps = o_psp.tile([C, OT], F32, tag="o_ps")
                b0 = i * OT
                nc.tensor.matmul(o_ps[:], w2a[:], x1[:, b0:b0 + OT], start=True, stop=False)
                nc.tensor.matmul(o_ps[:], wsc_r[0][:], xr[:, b0:b0 + OT], start=False, stop=False)
                nc.tensor.matmul(o_ps[:], wsc_r[1][:], xr[:, b0 + 1:b0 + 1 + OT], start=False, stop=False)
                nc.tensor.matmul(o_ps[:], wsc_r[2][:], xr[:, b0 + 2:b0 + 2 + OT], start=False, stop=True)
                ot = out_pool.tile([C, OT], F32, tag="ot")
                nc.scalar.activation(ot[:], o_ps[:], AF.Relu, bias=t2_t[:], scale=1.0)
                nc.sync.dma_start(out_ap[b, :, b0:b0 + OT], ot[:])

    nc.compile()
    _BUILD_CACHE[reps] = nc
    return nc


def _host_weights(w_fc1, b_fc1, bn1_g, bn1_b, bn1_rm, bn1_rv, w_fc2, b_fc2,
                  w1, w2, w_sp, w_sc, bn2_g, bn2_b, bn2_rm, bn2_rv):
    f = np.float32
    s1 = (bn1_g / np.sqrt(bn1_rv + EPS)).astype(f)
    t1 = (bn1_b - bn1_rm * s1).astype(f)
    wfc1 = np.ascontiguousarray(((w_fc1 * s1[:, None]) / L).T, dtype=f)      # [C, CQ]
    b1e = np.ascontiguousarray((b_fc1 * s1 + t1)[:, None], dtype=f)          # [CQ, 1]
    wfc2 = np.ascontiguousarray(w_fc2.T, dtype=f)                            # [CQ, C]
    b2 = np.ascontiguousarray(b_fc2[:, None], dtype=f)                       # [C, 1]
    w_u = np.ascontiguousarray((np.eye(C, dtype=f) + w1[:, :, 0]).T, dtype=f)
    w2t = np.ascontiguousarray(w2[:, :, 0].T, dtype=f)
    s2 = (bn2_g / np.sqrt(bn2_rv + EPS)).astype(f)
    t2 = np.ascontiguousarray((bn2_b - bn2_rm * s2)[:, None], dtype=f)
    wsc = [np.ascontiguousarray((w_sc[:, :, k] * s2[:, None]).T, dtype=f) for k in range(3)]
    taps = np.concatenate([w_sp[0, 0, :] / L, w_sp[0, 1, :]]).astype(f)[None, :]
    ident = np.eye(C, dtype=f)
    return {
        "w_u": w_u, "wsc0": wsc[0], "wsc1": wsc[1], "wsc2": wsc[2],
        "w2t": w2t, "wfc1": wfc1, "b1e": b1e, "wfc2": wfc2, "b2": b2,
        "t2": t2, "ident": ident, "taps": np.ascontiguousarray(taps),
    }


def kernel(x, w_fc1, b_fc1, bn1_g, bn1_b, bn1_rm, bn1_rv, w_fc2, b_fc2,
           w1, w2, w_sp, w_sc, bn2_g, bn2_b, bn2_rm, bn2_rv):
    x = np.asarray(x, dtype=np.float32)
    wd = _host_weights(np.asarray(w_fc1, np.float32), np.asarray(b_fc1, np.float32),
                       np.asarray(bn1_g, np.float32), np.asarray(bn1_b, np.float32),
                       np.asarray(bn1_rm, np.float32), np.asarray(bn1_rv, np.float32),
                       np.asarray(w_fc2, np.float32), np.asarray(b_fc2, np.float32),
                       np.asarray(w1, np.float32), np.asarray(w2, np.float32),
                       np.asarray(w_sp, np.float32), np.asarray(w_sc, np.float32),
                       np.asarray(bn2_g, np.float32), np.asarray(bn2_b, np.float32),
                       np.asarray(bn2_rm, np.float32), np.asarray(bn2_rv, np.float32))

    nc = _build()
    in_maps = []
    for c in range(N_CORES):
        m = dict(wd)
        m["x_dram"] = np.ascontiguousarray(x[c * NB:(c + 1) * NB])
        in_maps.append(m)
    res = bass_utils.run_bass_kernel_spmd(nc, in_maps, core_ids=list(range(N_CORES)))
    out = np.concatenate([res.results[c]["out_dram"] for c in range(N_CORES)], axis=0)
    return out.astype(np.float32)


# revision 11
# speedup vs baseline: 211.3100x; 211.3100x over previous
import sys

if "/opt/trn_rl_repo" not in sys.path:
    sys.path.insert(0, "/opt/trn_rl_repo")

import numpy as np
from contextlib import ExitStack

import concourse.tile as tile
from concourse import bacc, mybir
from concourse import bass_utils

F32 = mybir.dt.float32
F32R = mybir.dt.float32r
AF = mybir.ActivationFunctionType
ALU = mybir.AluOpType
AX = mybir.AxisListType

B, C, L = 32, 128, 8192
N_CORES = 8
NB = B // N_CORES          # batches per core
CQ = C // 4
EPS = 1e-5
CH = 2048                  # P1 chunk
PCH = 1024                 # P2 chunk (2 PSUM banks)
OT = 512                   # P3 out tile (1 PSUM bank)

_BUILD_CACHE = {}


def _build(reps=1, loop_reps=0):
    key = (reps, loop_reps)
    if key in _BUILD_CACHE:
        return _BUILD_CACHE[key]

    nc = bacc.Bacc("TRN2", target_bir_lowering=False, debug=False)

    x_ap = nc.dram_tensor("x_dram", [NB, C, L], F32R, kind="ExternalInput").ap()
    w_u_ap = nc.dram_tensor("w_u", [C, C], F32, kind="ExternalInput").ap()
    wsc_aps = [nc.dram_tensor(f"wsc{k}", [C, C], F32, kind="ExternalInput").ap() for k in range(3)]
    w2t_ap = nc.dram_tensor("w2t", [C, C], F32, kind="ExternalInput").ap()
    wfc1_ap = nc.dram_tensor("wfc1", [C, CQ], F32, kind="ExternalInput").ap()
    b1e_ap = nc.dram_tensor("b1e", [CQ, 1], F32, kind="ExternalInput").ap()
    wfc2_ap = nc.dram_tensor("wfc2", [CQ, C], F32, kind="ExternalInput").ap()
    b2_ap = nc.dram_tensor("b2", [C, 1], F32, kind="ExternalInput").ap()
    t2_ap = nc.dram_tensor("t2", [C, 1], F32, kind="ExternalInput").ap()
    wam_ap = nc.dram_tensor("wam", [C, C], F32, kind="ExternalInput").ap()
    wax_ap = nc.dram_tensor("wax", [C, C], F32, kind="ExternalInput").ap()
    ident_ap = nc.dram_tensor("ident", [C, C], F32, kind="ExternalInput").ap()
    out_ap = nc.dram_tensor("out_dram", [NB, C, L], F32, kind="ExternalOutput").ap()

    with tile.TileContext(nc) as tc, ExitStack() as ctx:
        wpool = ctx.enter_context(tc.tile_pool(name="wpool", bufs=1))
        xr_pool = ctx.enter_context(tc.tile_pool(name="xr", bufs=3))
        x1_pool = ctx.enter_context(tc.tile_pool(name="x1", bufs=2))
        scr_pool = ctx.enter_context(tc.tile_pool(name="scr", bufs=2))
        m_pool = ctx.enter_context(tc.tile_pool(name="mtile", bufs=2))
        out_pool = ctx.enter_context(tc.tile_pool(name="ot", bufs=3))
        st_pool = ctx.enter_context(tc.tile_pool(name="stats", bufs=2))
        row_pool = ctx.enter_context(tc.tile_pool(name="rows", bufs=2))
        w2a_pool = ctx.enter_context(tc.tile_pool(name="w2a", bufs=2))
        u_psp = ctx.enter_context(tc.tile_pool(name="u_ps", bufs=2, space="PSUM"))
        o_psp = ctx.enter_context(tc.tile_pool(name="o_ps", bufs=2, space="PSUM"))
        s_psp = ctx.enter_context(tc.tile_pool(name="s_ps", bufs=2, space="PSUM"))

        # ---- load + prep weights (once) ----
        def wload(nm, ap, shape):
            t = wpool.tile(shape, F32, tag=nm)
            nc.sync.dma_start(t[:], ap[:])
            return t

        w_u_f = wload("w_u_f", w_u_ap, [C, C])
        wsc_f = [wload(f"wsc{k}_f", wsc_aps[k], [C, C]) for k in range(3)]
        w2t_t = wload("w2t_t", w2t_ap, [C, C])
        wfc1_t = wload("wfc1_t", wfc1_ap, [C, CQ])
        b1e_t = wload("b1e_t", b1e_ap, [CQ, 1])
        wfc2_t = wload("wfc2_t", wfc2_ap, [CQ, C])
        b2_t = wload("b2_t", b2_ap, [C, 1])
        t2_t = wload("t2_t", t2_ap, [C, 1])
        wam_t = wload("wam_t", wam_ap, [C, C])
        wax_t = wload("wax_t", wax_ap, [C, C])
        ident_t = wload("ident_t", ident_ap, [C, C])

        w_u_r = wpool.tile([C, C], F32R, tag="w_u_r")
        nc.vector.tensor_scalar(w_u_r[:], w_u_f[:], 0.0, None, ALU.add)
        wsc_r = []
        for k in range(3):
            t = wpool.tile([C, C], F32R, tag=f"wsc{k}_r")
            nc.vector.tensor_scalar(t[:], wsc_f[k][:], 0.0, None, ALU.add)
            wsc_r.append(t)
        ones_t = wpool.tile([1, C], F32, tag="ones_t")
        nc.vector.memset(ones_t[:], 1.0)

        # ---- per-batch pipeline, software-pipelined depth 2 ----
        loop_cm = tc.For_i(0, loop_reps, 1) if loop_reps else None
        if loop_cm is not None:
            loop_cm.__enter__()

        def p1(b, st):
            xr = xr_pool.tile([C, L + 2], F32R, tag="xr")
            st["xr"] = xr
            nc.vector.memset(xr[:, 0:1].bitcast(F32), 0.0)
            nc.vector.memset(xr[:, L + 1:L + 2].bitcast(F32), 0.0)
            sabs_p = st_pool.tile([C, L // CH], F32, tag="sabs_p")
            st["sabs_p"] = sabs_p
            for q in range(L // CH):
                xsl = xr[:, 1 + q * CH:1 + (q + 1) * CH]
                nc.sync.dma_start(xsl, x_ap[b, :, q * CH:(q + 1) * CH])
                scr = scr_pool.tile([C, CH], F32, tag="scr")
                nc.scalar.activation(scr[:], xsl.bitcast(F32), AF.Abs,
                                     accum_out=sabs_p[:, q:q + 1])

        def mlp(b, st):
            sabs = st_pool.tile([C, 1], F32, tag="sabs")
            nc.vector.tensor_reduce(sabs[:], st["sabs_p"][:], AX.X, ALU.add)
            h_ps = s_psp.tile([CQ, 1], F32, tag="s_ps")
            nc.tensor.matmul(h_ps[:], wfc1_t[:], sabs[:], start=True, stop=True)
            h_t = st_pool.tile([CQ, 1], F32, tag="h_t")
            nc.scalar.activation(h_t[:], h_ps[:], AF.Relu, bias=b1e_t[:], scale=1.0)
            y_ps = s_psp.tile([C, 1], F32, tag="s_ps")
            nc.tensor.matmul(y_ps[:], wfc2_t[:], h_t[:], start=True, stop=True)
            x12 = st_pool.tile([C, 1], F32, tag="x12")
            nc.scalar.activation(x12[:], y_ps[:], AF.Sigmoid, bias=b2_t[:], scale=1.0)
            tpos = st_pool.tile([C, 1], F32, tag="tpos")
            nc.vector.scalar_tensor_tensor(tpos[:], sabs[:], 1.0 / L, x12[:], ALU.mult, ALU.mult)
            negt = st_pool.tile([C, 1], F32, tag="negt")
            nc.vector.scalar_tensor_tensor(negt[:], sabs[:], -1.0 / L, x12[:], ALU.mult, ALU.mult)
            st["tpos"], st["negt"] = tpos, negt

        def p2(b, st):
            xr, tpos, negt = st["xr"], st["tpos"], st["negt"]
            x1 = x1_pool.tile([C, L], F32R, tag="x1")
            st["x1"] = x1
            ssum_p = st_pool.tile([C, L // PCH], F32, tag="ssum_p")
            st["ssum_p"] = ssum_p
            for p in range(L // PCH):
                u_ps = u_psp.tile([C, PCH], F32, tag="u_ps")
                base = 1 + p * PCH
                nc.tensor.matmul(u_ps[:, 0:512], w_u_r[:], xr[:, base:base + 512],
                                 start=True, stop=True)
                nc.tensor.matmul(u_ps[:, 512:1024], w_u_r[:], xr[:, base + 512:base + 1024],
                                 start=True, stop=True)
                m_t = m_pool.tile([C, PCH], F32, tag="m_t")
                nc.vector.scalar_tensor_tensor(m_t[:], u_ps[:], tpos[:], xr[:, base:base + PCH],
                                               ALU.add, ALU.min)
                nc.vector.scalar_tensor_tensor(x1[:, p * PCH:(p + 1) * PCH], u_ps[:], negt[:],
                                               m_t[:], ALU.add, ALU.max,
                                               accum_out=ssum_p[:, p:p + 1])
            smax_p = st_pool.tile([C, L // CH], F32, tag="smax_p")
            st["smax_p"] = smax_p
            for q in range(L // CH):
                scr2 = scr_pool.tile([C, CH], F32, tag="scr")
                nc.vector.tensor_scalar(scr2[:], x1[:, q * CH:(q + 1) * CH], 0.0, None,
                                        ALU.add, ALU.max, accum_out=smax_p[:, q:q + 1])

        def ach(b, st):
            s_x1 = st_pool.tile([C, 1], F32, tag="s_x1")
            nc.vector.tensor_reduce(s_x1[:], st["ssum_p"][:], AX.X, ALU.add)
            mx = st_pool.tile([C, 1], F32, tag="mx")
            nc.vector.tensor_reduce(mx[:], st["smax_p"][:], AX.X, ALU.max)
            lg_ps = s_psp.tile([C, 1], F32, tag="s_ps")
            nc.tensor.matmul(lg_ps[:], wam_t[:], s_x1[:], start=True, stop=False)
            nc.tensor.matmul(lg_ps[:], wax_t[:], mx[:], start=False, stop=True)
            acol = st_pool.tile([C, 1], F32, tag="acol")
            nc.scalar.activation(acol[:], lg_ps[:], AF.Sigmoid)
            ar_ps = s_psp.tile([1, C], F32, tag="s_ps")
            nc.tensor.transpose(ar_ps[:], acol[:], ident_t[:])
            arow = row_pool.tile([1, C], F32, tag="arow")
            nc.vector.tensor_copy(arow[:], ar_ps[:])
            bc_ps = s_psp.tile([C, C], F32, tag="s_ps")
            nc.tensor.matmul(bc_ps[:], ones_t[:], arow[:], start=True, stop=True)
            w2a = w2a_pool.tile([C, C], F32R, tag="w2a")
            nc.vector.tensor_tensor(w2a[:], w2t_t[:], bc_ps[:], ALU.mult)
            st["w2a"] = w2a

        def p3(b, st):
            xr, x1, w2a = st["xr"], st["x1"], st["w2a"]
            for i in range(L // OT):
                o_ps = o_psp.tile([C, OT], F32, tag="o_ps")
                b0 = i * OT
                nc.tensor.matmul(o_ps[:], w2a[:], x1[:, b0:b0 + OT], start=True, stop=False)
                nc.tensor.matmul(o_ps[:], wsc_r[0][:], xr[:, b0:b0 + OT], start=False, stop=False)
                nc.tensor.matmul(o_ps[:], wsc_r[1][:], xr[:, b0 + 1:b0 + 1 + OT], start=False, stop=False)
                nc.tensor.matmul(o_ps[:], wsc_r[2][:], xr[:, b0 + 2:b0 + 2 + OT], start=False, stop=True)
                ot = out_pool.tile([C, OT], F32, tag="ot")
                nc.scalar.activation(ot[:], o_ps[:], AF.Relu, bias=t2_t[:], scale=1.0)
                nc.sync.dma_start(out_ap[b, :, b0:b0 + OT], ot[:])

        seq = [b for _ in range(reps) for b in range(NB)]
        states = {}
        for s in range(len(seq) + 2):
            if s < len(seq):
                states[s] = {}
                p1(seq[s], states[s])
            if 1 <= s <= len(seq):
                j = s - 1
                mlp(seq[j], states[j])
                p2(seq[j], states[j])
            if 2 <= s:
                j = s - 2
                ach(seq[j], states[j])
                p3(seq[j], states[j])
                del states[j]

        if loop_cm is not None:
            loop_cm.__exit__(None, None, None)

    nc.compile()
    _BUILD_CACHE[key] = nc
    return nc


def _host_weights(w_fc1, b_fc1, bn1_g, bn1_b, bn1_rm, bn1_rv, w_fc2, b_fc2,
                  w1, w2, w_sp, w_sc, bn2_g, bn2_b, bn2_rm, bn2_rv):
    f = np.float32
    s1 = (bn1_g / np.sqrt(bn1_rv + EPS)).astype(f)
    t1 = (bn1_b - bn1_rm * s1).astype(f)
    wfc1 = np.ascontiguousarray(((w_fc1 * s1[:, None]) / L).T, dtype=f)      # [C, CQ]
    b1e = np.ascontiguousarray((b_fc1 * s1 + t1)[:, None], dtype=f)          # [CQ, 1]
    wfc2 = np.ascontiguousarray(w_fc2.T, dtype=f)                            # [CQ, C]
    b2 = np.ascontiguousarray(b_fc2[:, None], dtype=f)                       # [C, 1]
    w_u = np.ascontiguousarray((np.eye(C, dtype=f) + w1[:, :, 0]).T, dtype=f)
    w2t = np.ascontiguousarray(w2[:, :, 0].T, dtype=f)
    s2 = (bn2_g / np.sqrt(bn2_rv + EPS)).astype(f)
    t2 = np.ascontiguousarray((bn2_b - bn2_rm * s2)[:, None], dtype=f)
    wsc = [np.ascontiguousarray((w_sc[:, :, k] * s2[:, None]).T, dtype=f) for k in range(3)]
    # banded matrices for the channel-axis conv of [mean, max] rows:
    # logit[c] = sum_k wm_k mean[c+k-1] + sum_k wx_k max[c+k-1]  (zero-padded)
    wm = (w_sp[0, 0, :] / L).astype(f)
    wx = w_sp[0, 1, :].astype(f)
    am = (wm[0] * np.eye(C, k=-1) + wm[1] * np.eye(C) + wm[2] * np.eye(C, k=1)).astype(f)
    ax = (wx[0] * np.eye(C, k=-1) + wx[1] * np.eye(C) + wx[2] * np.eye(C, k=1)).astype(f)
    ident = np.eye(C, dtype=f)
    return {
        "w_u": w_u, "wsc0": wsc[0], "wsc1": wsc[1], "wsc2": wsc[2],
        "w2t": w2t, "wfc1": wfc1, "b1e": b1e, "wfc2": wfc2, "b2": b2,
        "t2": t2, "ident": ident,
        "wam": np.ascontiguousarray(am.T), "wax": np.ascontiguousarray(ax.T),
    }


def kernel(x, w_fc1, b_fc1, bn1_g, bn1_b, bn1_rm, bn1_rv, w_fc2, b_fc2,
           w1, w2, w_sp, w_sc, bn2_g, bn2_b, bn2_rm, bn2_rv):
    x = np.asarray(x, dtype=np.float32)
    wd = _host_weights(np.asarray(w_fc1, np.float32), np.asarray(b_fc1, np.float32),
                       np.asarray(bn1_g, np.float32), np.asarray(bn1_b, np.float32),
                       np.asarray(bn1_rm, np.float32), np.asarray(bn1_rv, np.float32),
                       np.asarray(w_fc2, np.float32), np.asarray(b_fc2, np.float32),
                       np.asarray(w1, np.float32), np.asarray(w2, np.float32),
                       np.asarray(w_sp, np.float32), np.asarray(w_sc, np.float32),
                       np.asarray(bn2_g, np.float32), np.asarray(bn2_b, np.float32),
                       np.asarray(bn2_rm, np.float32), np.asarray(bn2_rv, np.float32))

    nc = _build()
    in_maps = []
    for c in range(N_CORES):
        m = dict(wd)
        m["x_dram"] = np.ascontiguousarray(x[c * NB:(c + 1) * NB])
        in_maps.append(m)
    res = bass_utils.run_bass_kernel_spmd(nc, in_maps, core_ids=list(range(N_CORES)))
    out = np.concatenate([res.results[c]["out_dram"] for c in range(N_CORES)], axis=0)
    return out.astype(np.float32)
